# revision 17
# baseline (speedup 1.0000x reference)
"""Trainium2 Bass kernel for nn_EnhancedTransformer_15350213116361.

Sequence-parallel across 8 NeuronCores: core c owns positions
[256c, 256c+256) of ALL batches (rows r = b*256 + s_local, 2048 rows/core).
Window attention (W=64) is position-local, FFN/LN/projections are row-local,
and the cross-batch interaction MHA needs only same-position rows, which a
sequence shard keeps together.  The only cross-core data dependency is the
per-batch Gram matrix M_b = sn_b^T tn_b (summed over all S), handled with a
single 512 KB AllReduce.

Layout: rows on partitions, [128p, 16 tiles, chan] sbuf tensors; matmuls are
fed by PE-transposed stationaries or transposed weights; compute in bf16
(tolerance gate is 2e-2), accumulation in f32.
"""

import sys

sys.path.insert(0, "/opt/trn_rl_repo")

import numpy as np
import ml_dtypes

import concourse.bass as bass
import concourse.bacc as bacc
import concourse.tile as tile
from concourse import mybir
from concourse.bass_utils import run_bass_kernel_spmd

BF16 = mybir.dt.bfloat16
F32 = mybir.dt.float32
AF = mybir.ActivationFunctionType
OP = mybir.AluOpType

B, S, D, H, W = 8, 2048, 128, 8, 64
NCORE = 8
SP = S // NCORE          # positions per core
R = B * SP               # rows per core
T = R // 128             # 128-row tiles per core
EPS_LN = 1e-5
EPS_COS = 1e-8

_BUILD_CACHE = {}
_last_in_maps = None


def _build(n_cores: int, stage: float = 99):
    """Build the per-core SPMD Bass program (same program on every core).

    `stage` truncates the program for bisection (99 = full kernel).
    """
    nc = bacc.Bacc(None, target_bir_lowering=False)

    # ---- DRAM I/O ------------------------------------------------------
    xc_d = nc.dram_tensor("xc", [R, D], F32, kind="ExternalInput")
    spc_d = nc.dram_tensor("spc", [R, D], F32, kind="ExternalInput")
    tec_d = nc.dram_tensor("tec", [R, D], F32, kind="ExternalInput")

    def win(name, shape, dt=BF16):
        return nc.dram_tensor(name, shape, dt, kind="ExternalInput")

    wqte_d = win("wqte", [D, D])
    wqto_d = win("wqto", [D, D])
    wkte_d = win("wkte", [D, D])
    wkto_d = win("wkto", [D, D])
    wvaug_d = win("wvaug", [D, 8 * 17])
    wout_d = win("woutT", [D, D])
    w1t_d = win("w1t", [D, 4, D])
    w2t_d = win("w2t", [D, 4, D])
    wsq_d = win("wsqT", [D, 2 * D])
    wtkv_d = win("wtkvT", [D, 3 * D])
    wsp_d = win("wsp", [D, D])
    woit_d = win("woiT", [D, D])
    mask_d = win("mask01T", [D, 8 * W])
    identb_d = win("identb", [D, D])
    identf_d = win("identf", [D, D], F32)

    out_d = nc.dram_tensor("outc", [R, D], F32, kind="ExternalOutput")

    xc_t = xc_d.rearrange("(t p) d -> p t d", p=128)
    spc_t = spc_d.rearrange("(t p) d -> p t d", p=128)
    tec_t = tec_d.rearrange("(t p) d -> p t d", p=128)
    out_t = out_d.rearrange("(t p) d -> p t d", p=128)

    with tile.TileContext(nc) as tc:
        with (
            tc.tile_pool(name="big", bufs=1) as big,          # persistent sbuf
            tc.tile_pool(name="wts", bufs=1) as wts,          # weights/constants
            tc.tile_pool(name="rot", bufs=3) as rot,          # rotating sbuf tiles
            tc.tile_pool(name="st", bufs=4) as st,            # small stats tiles
            tc.tile_pool(name="p512", bufs=2, space="PSUM") as p512,
            tc.tile_pool(name="psm", bufs=3, space="PSUM") as psm,
            tc.tile_pool(name="pmp", bufs=1, space="PSUM") as pmp,
            tc.tile_pool(name="dram", bufs=1, space="DRAM") as dram,
        ):
            # ---- weight / const loads ---------------------------------
            def wtile(dram_t, shape, dt=BF16):
                t_ = wts.tile(shape, dt, tag=dram_t.name)
                nc.sync.dma_start(out=t_[:], in_=dram_t[:])
                return t_

            wqte = wtile(wqte_d, [D, D])
            wqto = wtile(wqto_d, [D, D])
            wkte = wtile(wkte_d, [D, D])
            wkto = wtile(wkto_d, [D, D])
            wvaug = wtile(wvaug_d, [D, 8 * 17])
            woutT = wtile(wout_d, [D, D])
            w1t = wtile(w1t_d, [D, 4, D])
            w2t = wtile(w2t_d, [D, 4, D])
            wsqT = wtile(wsq_d, [D, 2 * D])
            wtkvT = wtile(wtkv_d, [D, 3 * D])
            wsp = wtile(wsp_d, [D, D])
            woiT = wtile(woit_d, [D, D])
            mask01 = wtile(mask_d, [D, 8 * W])
            identb = wtile(identb_d, [D, D])
            identf = wtile(identf_d, [D, D], F32)

            epsln = wts.tile([128, 1], F32, tag="epsln")
            nc.vector.memset(epsln[:], EPS_LN)

            # ---- input loads ------------------------------------------
            x_f = big.tile([128, T, D], F32, tag="x_f")
            sp_f = big.tile([128, T, D], F32, tag="sp_f")
            te_raw = big.tile([128, T, D], F32, tag="te_raw")
            for q in range(4):
                sl = slice(4 * q, 4 * q + 4)
                nc.sync.dma_start(out=x_f[:, sl, :], in_=xc_t[:, sl, :])
                nc.sync.dma_start(out=sp_f[:, sl, :], in_=spc_t[:, sl, :])
                nc.sync.dma_start(out=te_raw[:, sl, :], in_=tec_t[:, sl, :])

            def section(n):
                return stage >= n

            # ---- transposes of x / spatial / temporal -----------------
            if section(2):
                xT = big.tile([128, T, D], BF16, tag="xT")
                spT = big.tile([128, T, D], BF16, tag="spT")
                teT = big.tile([128, T, D], BF16, tag="teT")
                for t in range(T):
                    for src, dst in ((x_f, xT), (sp_f, spT), (te_raw, teT)):
                        ps = psm.tile([128, D], F32, tag="sm")
                        nc.tensor.transpose(ps[:], src[:, t, :], identf[:])
                        nc.any.tensor_copy(out=dst[:, t, :], in_=ps[:])

            # ---- padded q/k projections (W-stationary, xT moving) -----
            if section(3):
                qeT = big.tile([128, T, D], BF16, tag="qeT")
                qoT = big.tile([128, T, D], BF16, tag="qoT")
                keT = big.tile([128, T, D], BF16, tag="keT")
                koT = big.tile([128, T, D], BF16, tag="koT")
                for wmat, dst in ((wqte, qeT), (wqto, qoT), (wkte, keT), (wkto, koT)):
                    for q in range(4):
                        sl = slice(4 * q, 4 * q + 4)
                        ps = p512.tile([128, 512], F32, tag="A")
                        nc.tensor.matmul(ps[:], wmat[:], xT[:, sl, :])
                        nc.any.tensor_copy(
                            out=dst[:, sl, :],
                            in_=ps[:].rearrange("p (a b) -> p a b", a=4),
                        )

            # ---- v_aug projection (xT tile stationary) ----------------
            if section(4):
                vaug = big.tile([128, T, 8 * 17], BF16, tag="vaug")
                for t in range(T):
                    ps = psm.tile([128, 8 * 17], F32, tag="sm")
                    nc.tensor.matmul(ps[:], xT[:, t, :], wvaug[:])
                    nc.any.tensor_copy(out=vaug[:, t, :], in_=ps[:])
                    nc.vector.memset(
                        vaug[:, t, :].rearrange("p (h c) -> p h c", c=17)[:, :, 16],
                        1.0,
                    )

            # ---- se|q_int and te|k_int|v_int projections --------------
            if section(5):
                se_f = big.tile([128, T, D], F32, tag="se_f")
                tee_f = big.tile([128, T, D], F32, tag="tee_f")
                q_int = big.tile([128, T, D], BF16, tag="q_int")
                k_int = big.tile([128, T, D], BF16, tag="k_int")
                v_int = big.tile([128, T, D], BF16, tag="v_int")
                for t in range(T):
                    ps = p512.tile([128, 2 * D], F32, tag="A")
                    nc.tensor.matmul(ps[:], spT[:, t, :], wsqT[:])
                    nc.any.tensor_copy(out=se_f[:, t, :], in_=ps[:, 0:D])
                    nc.any.tensor_copy(out=q_int[:, t, :], in_=ps[:, D : 2 * D])
                    ps2 = p512.tile([128, 3 * D], F32, tag="A")
                    nc.tensor.matmul(ps2[:], teT[:, t, :], wtkvT[:])
                    nc.any.tensor_copy(out=tee_f[:, t, :], in_=ps2[:, 0:D])
                    nc.any.tensor_copy(out=k_int[:, t, :], in_=ps2[:, D : 2 * D])
                    nc.any.tensor_copy(out=v_int[:, t, :], in_=ps2[:, 2 * D : 3 * D])

            # ---- cosine norms, sn/tn, M partials ----------------------
            if section(6):
                rsn = big.tile([128, T], F32, tag="rsn")
                rtn = big.tile([128, T], F32, tag="rtn")
                m_sb = big.tile([128, B, D], F32, tag="m_sb")
                mps_hold = None
                for t in range(T):
                    b = t // 2
                    sntn = []
                    for src, rdst in ((se_f, rsn), (tee_f, rtn)):
                        sq = st.tile([128, D], BF16, tag="sq")
                        ssq = st.tile([128, 1], F32, tag="ssq")
                        nc.scalar.activation(
                            out=sq[:], in_=src[:, t, :], func=AF.Square,
                            accum_out=ssq[:],
                        )
                        nrm = st.tile([128, 1], F32, tag="nrm")
                        nc.scalar.sqrt(out=nrm[:], in_=ssq[:])
                        nc.vector.tensor_scalar_max(
                            out=nrm[:], in0=nrm[:], scalar1=EPS_COS
                        )
                        nc.vector.reciprocal(out=rdst[:, t : t + 1], in_=nrm[:])
                        nt = rot.tile([128, D], BF16, tag="sntn")
                        nc.vector.tensor_scalar_mul(
                            out=nt[:], in0=src[:, t, :], scalar1=rdst[:, t : t + 1]
                        )
                        sntn.append(nt)
                    if t % 2 == 0:
                        mps_hold = pmp.tile([128, D], F32, tag="mp")
                    nc.tensor.matmul(
                        mps_hold[:], sntn[0][:], sntn[1][:],
                        start=(t % 2 == 0), stop=(t % 2 == 1),
                    )
                    if t % 2 == 1:
                        nc.any.tensor_copy(out=m_sb[:, b, :], in_=mps_hold[:])

            # ---- AllReduce of M ---------------------------------------
            if section(7):
                m_in = dram.tile([B * D, D], F32, tag="m_in")
                m_out = dram.tile([B * D, D], F32, tag="m_out")
                nc.gpsimd.dma_start(
                    out=m_in[:].rearrange("(b p) d -> p b d", p=128), in_=m_sb[:]
                )
                nc.gpsimd.collective_compute(
                    "AllReduce",
                    OP.add,
                    replica_groups=[list(range(n_cores))],
                    ins=[m_in[:].opt()],
                    outs=[m_out[:].opt()],
                )
                m_all = big.tile([128, B, D], F32, tag="m_all")
                nc.gpsimd.dma_start(
                    out=m_all[:], in_=m_out[:].rearrange("(b p) d -> p b d", p=128)
                )
                m_bf = big.tile([128, B, D], BF16, tag="m_bf")
                for b in range(B):
                    nc.scalar.mul(out=m_bf[:, b, :], in_=m_all[:, b, :], mul=1.0 / S)
                g_bf = big.tile([128, B, D], BF16, tag="g_bf")
                for b in range(B):
                    ps = psm.tile([128, D], F32, tag="sm")
                    nc.tensor.matmul(ps[:], wsp[:], m_bf[:, b, :])
                    nc.any.tensor_copy(out=g_bf[:, b, :], in_=ps[:])

            # ---- local window attention + LN1 -------------------------
            if section(8):
                ln1 = big.tile([128, T, D], F32, tag="ln1")
                ln1T = big.tile([128, T, D], BF16, tag="ln1T")
                for t in range(T):
                    # one psum tile per chan-strip s4 (row group): 4 MMs each
                    # (par x window), same row-strip -> bank sharing is safe
                    sc4 = []
                    for s4 in range(4):
                        scp = p512.tile([128, 128], F32, tag="A", name=f"scp{s4}")
                        mm4 = [(par, wdw) for par in (0, 1) for wdw in (0, 1)]
                        for idx, (par, wdw) in enumerate(mm4):
                            qT_, kT_ = (qeT, keT) if par == 0 else (qoT, koT)
                            nc.tensor.matmul(
                                scp[64 * wdw : 64 * wdw + 64,
                                    64 * par : 64 * par + 64],
                                kT_[32 * s4 : 32 * s4 + 32, t,
                                    64 * wdw : 64 * wdw + 64],
                                qT_[32 * s4 : 32 * s4 + 32, t,
                                    64 * wdw : 64 * wdw + 64],
                                tile_position=(32 * s4, 64 * wdw),
                                start=(idx == 0), stop=(idx == len(mm4) - 1),
                                skip_group_check=True,
                            )
                        sc4.append(scp)
                    if stage < 8.1:
                        continue
                    e_sb = rot.tile([128, 8 * W], BF16, tag="esb")
                    for s4 in range(4):
                        nc.scalar.activation(
                            out=e_sb[:, 128 * s4 : 128 * s4 + 128],
                            in_=sc4[s4][:], func=AF.Exp, scale=0.25,
                        )
                    nc.vector.tensor_mul(e_sb[:], e_sb[:], mask01[:])
                    if stage < 8.4:
                        continue
                    avp = []
                    for wdw in (0, 1):
                        avw = psm.tile([128, 8 * 17], F32, tag="sm", name=f"avw{wdw}")
                        ws = slice(64 * wdw, 64 * wdw + 64)
                        for h in range(8):
                            # e_sb col of head h = 128*(h//2) + 64*(h%2)
                            ec = 128 * (h // 2) + 64 * (h % 2)
                            nc.tensor.matmul(
                                avw[ws, 17 * h : 17 * h + 17],
                                e_sb[ws, ec : ec + W],
                                vaug[ws, t, 17 * h : 17 * h + 17],
                                start=(h == 0), stop=(h == 7),
                                skip_group_check=True,
                            )
                        avp.append(avw)
                    if stage < 8.6:
                        continue
                    av_sb = rot.tile([128, D], BF16, tag="avsb")
                    rden = st.tile([128, 8], F32, tag="rden")
                    for wdw in (0, 1):
                        ws = slice(64 * wdw, 64 * wdw + 64)
                        avv = avp[wdw][:].rearrange("p (h c) -> p h c", c=17)
                        nc.vector.reciprocal(out=rden[ws, :], in_=avv[ws, :, 16])
                        for h in range(8):
                            nc.vector.tensor_scalar_mul(
                                out=av_sb[ws, 16 * h : 16 * h + 16],
                                in0=avv[ws, h, 0:16],
                                scalar1=rden[ws, h : h + 1],
                            )
                    if stage < 8.8:
                        continue
                    avt_ps = psm.tile([128, D], BF16, tag="sm")
                    nc.tensor.transpose(avt_ps[:], av_sb[:], identb[:])
                    avT = rot.tile([128, D], BF16, tag="avT")
                    nc.any.tensor_copy(out=avT[:], in_=avt_ps[:])
                    ops_ = psm.tile([128, D], F32, tag="sm")
                    nc.tensor.matmul(ops_[:], avT[:], woutT[:])
                    res = rot.tile([128, D], F32, tag="res")
                    nc.vector.tensor_add(res[:], ops_[:], x_f[:, t, :])
                    stats = st.tile([128, 6], F32, tag="bst")
                    nc.vector.bn_stats(out=stats[:], in_=res[:])
                    mv = st.tile([128, 2], F32, tag="bagg")
                    nc.vector.bn_aggr(out=mv[:], in_=stats[:])
                    stdv = st.tile([128, 1], F32, tag="stdv")
                    nc.scalar.activation(
                        out=stdv[:], in_=mv[:, 1:2], func=AF.Sqrt, bias=epsln[:]
                    )
                    nc.vector.reciprocal(out=stdv[:], in_=stdv[:])
                    nc.vector.tensor_scalar(
                        out=ln1[:, t, :], in0=res[:],
                        scalar1=mv[:, 0:1], scalar2=stdv[:],
                        op0=OP.subtract, op1=OP.mult,
                    )
                    psT = psm.tile([128, D], F32, tag="sm")
                    nc.tensor.transpose(psT[:], ln1[:, t, :], identf[:])
                    nc.any.tensor_copy(out=ln1T[:, t, :], in_=psT[:])

            # ---- FFN (hT-direct) + LN2 --------------------------------
            if section(9):
                hT = big.tile([128, 4, T, D], BF16, tag="hT")
                for k4 in range(4):
                    for t4 in range(4):
                        sl = slice(4 * t4, 4 * t4 + 4)
                        ps = p512.tile([128, 512], F32, tag="A")
                        nc.tensor.matmul(ps[:], w1t[:, k4, :], ln1T[:, sl, :])
                        nc.scalar.activation(
                            out=hT[:, k4, sl, :].rearrange("p a b -> p (a b)"),
                            in_=ps[:], func=AF.Gelu,
                        )
                xm2 = big.tile([128, T, D], F32, tag="xm2")
                for t4 in range(4):
                    sl = slice(4 * t4, 4 * t4 + 4)
                    ps = p512.tile([128, 512], F32, tag="A")
                    for k4 in range(4):
                        nc.tensor.matmul(
                            ps[:], w2t[:, k4, :], hT[:, k4, sl, :],
                            start=(k4 == 0), stop=(k4 == 3),
                        )
                    o2T = rot.tile([128, 512], BF16, tag="o2T")
                    nc.any.tensor_copy(out=o2T[:], in_=ps[:])
                    for tt in range(4):
                        t = 4 * t4 + tt
                        tps = psm.tile([128, D], BF16, tag="sm")
                        nc.tensor.transpose(
                            tps[:], o2T[:, 128 * tt : 128 * tt + 128], identb[:]
                        )
                        res = rot.tile([128, D], F32, tag="res")
                        nc.vector.tensor_add(res[:], tps[:], ln1[:, t, :])
                        stats = st.tile([128, 6], F32, tag="bst")
                        nc.vector.bn_stats(out=stats[:], in_=res[:])
                        mv = st.tile([128, 2], F32, tag="bagg")
                        nc.vector.bn_aggr(out=mv[:], in_=stats[:])
                        stdv = st.tile([128, 1], F32, tag="stdv")
                        nc.scalar.activation(
                            out=stdv[:], in_=mv[:, 1:2], func=AF.Sqrt, bias=epsln[:]
                        )
                        nc.vector.reciprocal(out=stdv[:], in_=stdv[:])
                        nc.vector.tensor_scalar(
                            out=xm2[:, t, :], in0=res[:],
                            scalar1=mv[:, 0:1], scalar2=stdv[:],
                            op0=OP.subtract, op1=OP.mult,
                        )

            # ---- interaction MHA over the batch axis (DVE) ------------
            if section(10):
                z_all = big.tile([128, 2, 8, 8, 8], F32, tag="z_all")
                for hi in range(2):
                    kv_view = k_int[:].rearrange(
                        "p (bb two) c -> p two bb c", two=2
                    )[:, hi]
                    for i in range(8):
                        qa = q_int[:, 2 * i + hi, :]
                        tmp = rot.tile([128, 8, D], BF16, tag="itmp")
                        nc.vector.tensor_tensor(
                            out=tmp[:],
                            in0=kv_view,
                            in1=bass.AP(
                                tensor=qa.tensor, offset=qa.offset,
                                ap=[list(qa.ap[0]), [0, 8], [1, D]],
                            ),
                            op=OP.mult,
                        )
                        zslice = z_all[:, hi, i, :, :]
                        zout = bass.AP(
                            tensor=zslice.tensor, offset=zslice.offset,
                            ap=[list(zslice.ap[0]), [1, 8], [8, 8]],
                        )
                        nc.vector.reduce_sum(
                            out=zout,
                            in_=tmp[:].rearrange("p j (h c) -> p j h c", c=16),
                            axis=mybir.AxisListType.X,
                        )
                e_all = big.tile([128, 2, 8, 8, 8], BF16, tag="e_all")
                nc.scalar.activation(
                    out=e_all[:].rearrange("p a b c d -> p (a b c d)"),
                    in_=z_all[:].rearrange("p a b c d -> p (a b c d)"),
                    func=AF.Exp, scale=0.25,
                )
                den = st.tile([128, 2, 8, 8], F32, tag="iden")
                nc.vector.reduce_sum(
                    out=den[:], in_=e_all[:], axis=mybir.AxisListType.X
                )
                nc.vector.reciprocal(
                    out=den[:].rearrange("p a b c -> p (a b c)"),
                    in_=den[:].rearrange("p a b c -> p (a b c)"),
                )
                en = big.tile([128, 2, 8, 8, 8], BF16, tag="en")
                dv = den[:].rearrange("p a b c -> p (a b c)")
                nc.vector.tensor_tensor(
                    out=en[:].rearrange("p a b c d -> p (a b c) d"),
                    in0=e_all[:].rearrange("p a b c d -> p (a b c) d"),
                    in1=bass.AP(
                        tensor=dv.tensor, offset=dv.offset,
                        ap=[list(dv.ap[0]), [1, 128], [0, 8]],
                    ),
                    op=OP.mult,
                )
                av_int = big.tile([128, T, D], BF16, tag="av_int")
                for hi in range(2):
                    v_view = v_int[:].rearrange(
                        "p (bb two) c -> p two bb c", two=2
                    )[:, hi]
                    for i in range(8):
                        asl = en[:, hi, i, :, :]
                        a_ap = bass.AP(
                            tensor=asl.tensor, offset=asl.offset,
                            ap=[list(asl.ap[0]), [1, 8], [8, 8], [0, 16]],
                        )
                        tmp = rot.tile([128, 8, D], BF16, tag="itmp")
                        nc.vector.tensor_tensor(
                            out=tmp[:],
                            in0=v_view.rearrange("p j (h c) -> p j h c", c=16),
                            in1=a_ap, op=OP.mult,
                        )
                        tv = tmp[:]
                        with nc.allow_low_precision(
                            reason="attn AV output rounds to bf16"
                        ):
                            nc.vector.reduce_sum(
                                out=av_int[:, 2 * i + hi, :],
                                in_=bass.AP(
                                    tensor=tv.tensor, offset=tv.offset,
                                    ap=[list(tv.ap[0]), [1, D], [D, 8]],
                                ),
                                axis=mybir.AxisListType.X,
                            )

            # ---- z / sim ----------------------------------------------
            if section(11):
                sim = big.tile([128, T], F32, tag="sim")
                for t in range(T):
                    b = t // 2
                    zps = psm.tile([128, D], F32, tag="sm")
                    nc.tensor.matmul(zps[:], spT[:, t, :], g_bf[:, b, :])
                    scratch = st.tile([128, D], F32, tag="zscr")
                    dot = st.tile([128, 1], F32, tag="zdot")
                    nc.vector.tensor_mul(scratch[:], zps[:], tee_f[:, t, :])
                    nc.vector.reduce_sum(
                        out=dot[:], in_=scratch[:], axis=mybir.AxisListType.X
                    )
                    nc.vector.tensor_scalar(
                        out=sim[:, t : t + 1], in0=dot[:],
                        scalar1=rsn[:, t : t + 1], scalar2=rtn[:, t : t + 1],
                        op0=OP.mult, op1=OP.mult,
                    )

            # ---- interaction out-proj + final combine -----------------
            if section(12):
                for t in range(T):
                    tps = psm.tile([128, D], BF16, tag="sm")
                    nc.tensor.transpose(tps[:], av_int[:, t, :], identb[:])
                    avIT = rot.tile([128, D], BF16, tag="avIT")
                    nc.any.tensor_copy(out=avIT[:], in_=tps[:])
                    ips = psm.tile([128, D], F32, tag="sm")
                    nc.tensor.matmul(ips[:], avIT[:], woiT[:])
                    outt = rot.tile([128, D], F32, tag="outt")
                    nc.vector.tensor_scalar_mul(
                        out=outt[:], in0=ips[:], scalar1=sim[:, t : t + 1]
                    )
                    nc.vector.tensor_add(outt[:], outt[:], xm2[:, t, :])
                    nc.sync.dma_start(out=out_t[:, t, :], in_=outt[:])

    nc.compile()
    return nc


def _prep_host(inputs):
    """Host-side weight folding / permutation. Returns dict of device arrays."""
    f32 = np.float32
    bf = ml_dtypes.bfloat16
    g = {k: np.asarray(v, f32) for k, v in inputs.items()}

    for nm in ("lw_in_b", "lw_out_b", "spat_b", "temp_b", "int_in_b",
               "int_out_b", "ffn_b1", "ffn_b2", "ln1_b", "ln2_b"):
        assert not np.any(g[nm]), f"nonzero bias {nm} unsupported"
    assert np.all(g["ln1_g"] == 1.0) and np.all(g["ln2_g"] == 1.0), "ln gamma"

    Wq, Wk, Wv = g["lw_in_w"][:D], g["lw_in_w"][D:2*D], g["lw_in_w"][2*D:]

    def padT(Wm, par):
        out = np.zeros((D, D), f32)
        for s4 in range(4):
            h = 2 * s4 + par
            out[32 * s4 : 32 * s4 + 16, :] = Wm[16 * h : 16 * h + 16, :]
        return np.ascontiguousarray(out.T)

    wvaug = np.zeros((D, 8 * 17), f32)
    for h in range(8):
        wvaug[:, 17 * h : 17 * h + 16] = Wv[16 * h : 16 * h + 16, :].T

    mask01 = np.zeros((D, 8 * W), f32)
    jj = np.arange(D) % W
    ii = np.arange(8 * W) % W
    mask01[:, :] = (jj[:, None] <= ii[None, :])

    WqI = g["int_in_w"][:D]
    WkI = g["int_in_w"][D:2*D]
    WvI = g["int_in_w"][2*D:]

    arrs = {
        "wqte": padT(Wq, 0), "wqto": padT(Wq, 1),
        "wkte": padT(Wk, 0), "wkto": padT(Wk, 1),
        "wvaug": wvaug,
        "woutT": g["lw_out_w"].T,
        "w1t": g["ffn_w1"].T.reshape(D, 4, D),
        "w2t": g["ffn_w2"].T.reshape(4, D, D).transpose(1, 0, 2),
        "wsqT": np.concatenate([g["spat_w"].T, (WqI @ g["spat_w"]).T], axis=1),
        "wtkvT": np.concatenate(
            [g["temp_w"].T, (WkI @ g["temp_w"]).T, (WvI @ g["temp_w"]).T], axis=1
        ),
        "wsp": g["spat_w"],
        "woiT": g["int_out_w"].T,
        "mask01T": mask01,
        "identb": np.eye(D, dtype=f32),
    }
    out = {k: np.ascontiguousarray(v.astype(bf)) for k, v in arrs.items()}
    out["identf"] = np.ascontiguousarray(np.eye(D, dtype=f32))
    return out


def kernel(x, spatial_info, temporal_info, **weights):
    global _last_in_maps
    inputs = dict(weights)
    x = np.ascontiguousarray(np.asarray(x, np.float32))
    sp = np.ascontiguousarray(np.asarray(spatial_info, np.float32))
    te = np.ascontiguousarray(np.asarray(temporal_info, np.float32))

    if "nc" not in _BUILD_CACHE:
        _BUILD_CACHE["nc"] = _build(NCORE)
    nc = _BUILD_CACHE["nc"]

    host = _prep_host(inputs)
    in_maps = []
    for c in range(NCORE):
        sl = slice(SP * c, SP * c + SP)
        m = dict(host)
        m["xc"] = np.ascontiguousarray(x[:, sl, :].reshape(R, D))
        m["spc"] = np.ascontiguousarray(sp[:, sl, :].reshape(R, D))
        m["tec"] = np.ascontiguousarray(te[:, sl, :].reshape(R, D))
        in_maps.append(m)
    _last_in_maps = in_maps

    res = run_bass_kernel_spmd(nc, in_maps, list(range(NCORE)))
    out = np.empty((B, S, D), np.float32)
    for c in range(NCORE):
        out[:, SP * c : SP * c + SP, :] = res.results[c]["outc"].reshape(B, SP, D)
    return out


# revision 20
# speedup vs baseline: 1.2618x; 1.2618x over previous
"""Trainium2 Bass kernel for nn_EnhancedTransformer_15350213116361.

Sequence-parallel across 8 NeuronCores: core c owns positions
[256c, 256c+256) of ALL batches (rows r = b*256 + s_local, 2048 rows/core).
Window attention (W=64) is position-local, FFN/LN/projections are row-local,
and the cross-batch interaction MHA needs only same-position rows, which a
sequence shard keeps together.  The only cross-core data dependency is the
per-batch Gram matrix M_b = sn_b^T tn_b (summed over all S), handled with a
single 512 KB AllReduce.

Layout: rows on partitions, [128p, 16 tiles, chan] sbuf tensors; matmuls are
fed by PE-transposed stationaries or transposed weights; compute in bf16
(tolerance gate is 2e-2), accumulation in f32.
"""

import sys

sys.path.insert(0, "/opt/trn_rl_repo")

import numpy as np
import ml_dtypes

import concourse.bass as bass
import concourse.bacc as bacc
import concourse.tile as tile
from concourse import mybir
from concourse.bass_utils import run_bass_kernel_spmd

BF16 = mybir.dt.bfloat16
F32 = mybir.dt.float32
AF = mybir.ActivationFunctionType
OP = mybir.AluOpType

B, S, D, H, W = 8, 2048, 128, 8, 64
NCORE = 8
SP = S // NCORE          # positions per core
R = B * SP               # rows per core
T = R // 128             # 128-row tiles per core
EPS_LN = 1e-5
EPS_COS = 1e-8

_BUILD_CACHE = {}
_last_in_maps = None


def _build(n_cores: int, stage: float = 99):
    """Build the per-core SPMD Bass program (same program on every core).

    `stage` truncates the program for bisection (99 = full kernel).
    """
    nc = bacc.Bacc(None, target_bir_lowering=False)

    # ---- DRAM I/O ------------------------------------------------------
    xc_d = nc.dram_tensor("xc", [R, D], F32, kind="ExternalInput")
    spc_d = nc.dram_tensor("spc", [R, D], F32, kind="ExternalInput")
    tec_d = nc.dram_tensor("tec", [R, D], F32, kind="ExternalInput")

    def win(name, shape, dt=BF16):
        return nc.dram_tensor(name, shape, dt, kind="ExternalInput")

    wqte_d = win("wqte", [D, D])
    wqto_d = win("wqto", [D, D])
    wkte_d = win("wkte", [D, D])
    wkto_d = win("wkto", [D, D])
    wvaug_d = win("wvaug", [D, 8 * 17])
    wout_d = win("woutT", [D, D])
    w1t_d = win("w1t", [D, 4, D])
    w2t_d = win("w2t", [D, 4, D])
    wsq_d = win("wsqT", [D, 2 * D])
    wtkv_d = win("wtkvT", [D, 3 * D])
    wsp_d = win("wsp", [D, D])
    woit_d = win("woiT", [D, D])
    mask_d = win("mask01T", [D, 8 * W])
    identb_d = win("identb", [D, D])
    identf_d = win("identf", [D, D], F32)

    out_d = nc.dram_tensor("outc", [R, D], F32, kind="ExternalOutput")

    xc_t = xc_d.rearrange("(t p) d -> p t d", p=128)
    spc_t = spc_d.rearrange("(t p) d -> p t d", p=128)
    tec_t = tec_d.rearrange("(t p) d -> p t d", p=128)
    out_t = out_d.rearrange("(t p) d -> p t d", p=128)

    with tile.TileContext(nc) as tc:
        with (
            tc.tile_pool(name="big", bufs=1) as big,          # persistent sbuf
            tc.tile_pool(name="wts", bufs=1) as wts,          # weights/constants
            tc.tile_pool(name="rot", bufs=3) as rot,          # rotating sbuf tiles
            tc.tile_pool(name="st", bufs=4) as st,            # small stats tiles
            tc.tile_pool(name="p512", bufs=2, space="PSUM") as p512,
            tc.tile_pool(name="psm", bufs=3, space="PSUM") as psm,
            tc.tile_pool(name="pmp", bufs=1, space="PSUM") as pmp,
            tc.tile_pool(name="dram", bufs=1, space="DRAM") as dram,
        ):
            # ---- weight / const loads ---------------------------------
            def wtile(dram_t, shape, dt=BF16):
                t_ = wts.tile(shape, dt, tag=dram_t.name)
                nc.sync.dma_start(out=t_[:], in_=dram_t[:])
                return t_

            wqte = wtile(wqte_d, [D, D])
            wqto = wtile(wqto_d, [D, D])
            wkte = wtile(wkte_d, [D, D])
            wkto = wtile(wkto_d, [D, D])
            wvaug = wtile(wvaug_d, [D, 8 * 17])
            woutT = wtile(wout_d, [D, D])
            w1t = wtile(w1t_d, [D, 4, D])
            w2t = wtile(w2t_d, [D, 4, D])
            wsqT = wtile(wsq_d, [D, 2 * D])
            wtkvT = wtile(wtkv_d, [D, 3 * D])
            wsp = wtile(wsp_d, [D, D])
            woiT = wtile(woit_d, [D, D])
            mask01 = wtile(mask_d, [D, 8 * W])
            identb = wtile(identb_d, [D, D])
            identf = wtile(identf_d, [D, D], F32)

            epsln = wts.tile([128, 1], F32, tag="epsln")
            nc.vector.memset(epsln[:], EPS_LN)

            # ---- input loads ------------------------------------------
            x_f = big.tile([128, T, D], F32, tag="x_f")
            sp_f = big.tile([128, T, D], F32, tag="sp_f")
            te_raw = big.tile([128, T, D], F32, tag="te_raw")
            for q in range(4):
                sl = slice(4 * q, 4 * q + 4)
                nc.sync.dma_start(out=x_f[:, sl, :], in_=xc_t[:, sl, :])
                nc.sync.dma_start(out=sp_f[:, sl, :], in_=spc_t[:, sl, :])
                nc.sync.dma_start(out=te_raw[:, sl, :], in_=tec_t[:, sl, :])

            def section(n):
                return stage >= n

            # ---- transposes of x / spatial / temporal -----------------
            if section(2):
                xT = big.tile([128, T, D], BF16, tag="xT")
                spT = big.tile([128, T, D], BF16, tag="spT")
                teT = big.tile([128, T, D], BF16, tag="teT")
                for t in range(T):
                    for src, dst in ((x_f, xT), (sp_f, spT), (te_raw, teT)):
                        ps = psm.tile([128, D], F32, tag="sm")
                        nc.tensor.transpose(ps[:], src[:, t, :], identf[:])
                        nc.any.tensor_copy(out=dst[:, t, :], in_=ps[:])

            # ---- padded q/k projections (W-stationary, xT moving) -----
            if section(3):
                qeT = big.tile([128, T, D], BF16, tag="qeT")
                qoT = big.tile([128, T, D], BF16, tag="qoT")
                keT = big.tile([128, T, D], BF16, tag="keT")
                koT = big.tile([128, T, D], BF16, tag="koT")
                for wmat, dst in ((wqte, qeT), (wqto, qoT), (wkte, keT), (wkto, koT)):
                    for q in range(4):
                        sl = slice(4 * q, 4 * q + 4)
                        ps = p512.tile([128, 512], F32, tag="A")
                        nc.tensor.matmul(ps[:], wmat[:], xT[:, sl, :])
                        nc.any.tensor_copy(
                            out=dst[:, sl, :],
                            in_=ps[:].rearrange("p (a b) -> p a b", a=4),
                        )

            # ---- v_aug projection (xT tile stationary) ----------------
            if section(4):
                vaug = big.tile([128, T, 8 * 17], BF16, tag="vaug")
                for t in range(T):
                    ps = psm.tile([128, 8 * 17], F32, tag="sm")
                    nc.tensor.matmul(ps[:], xT[:, t, :], wvaug[:])
                    nc.any.tensor_copy(out=vaug[:, t, :], in_=ps[:])
                    nc.vector.memset(
                        vaug[:, t, :].rearrange("p (h c) -> p h c", c=17)[:, :, 16],
                        1.0,
                    )

            # ---- se|q_int and te|k_int|v_int projections --------------
            if section(5):
                se_f = big.tile([128, T, D], F32, tag="se_f")
                tee_f = big.tile([128, T, D], F32, tag="tee_f")
                q_int = big.tile([128, T, D], BF16, tag="q_int")
                k_int = big.tile([128, T, D], BF16, tag="k_int")
                v_int = big.tile([128, T, D], BF16, tag="v_int")
                for t in range(T):
                    ps = p512.tile([128, 2 * D], F32, tag="A")
                    nc.tensor.matmul(ps[:], spT[:, t, :], wsqT[:])
                    nc.any.tensor_copy(out=se_f[:, t, :], in_=ps[:, 0:D])
                    nc.any.tensor_copy(out=q_int[:, t, :], in_=ps[:, D : 2 * D])
                    ps2 = p512.tile([128, 3 * D], F32, tag="A")
                    nc.tensor.matmul(ps2[:], teT[:, t, :], wtkvT[:])
                    nc.any.tensor_copy(out=tee_f[:, t, :], in_=ps2[:, 0:D])
                    nc.any.tensor_copy(out=k_int[:, t, :], in_=ps2[:, D : 2 * D])
                    nc.any.tensor_copy(out=v_int[:, t, :], in_=ps2[:, 2 * D : 3 * D])

            # ---- cosine norms, sn/tn, M partials ----------------------
            if section(6):
                rrn = big.tile([128, 2 * T], F32, tag="rrn")  # [rsn | rtn]
                m_sb = big.tile([128, B, D], F32, tag="m_sb")
                ssq_all = st.tile([128, 2 * T], F32, tag="ssq")
                for t in range(T):
                    for which, src in ((0, se_f), (1, tee_f)):
                        sq = st.tile([128, D], BF16, tag="sq")
                        nc.scalar.activation(
                            out=sq[:], in_=src[:, t, :], func=AF.Square,
                            accum_out=ssq_all[:, which * T + t : which * T + t + 1],
                        )
                nc.scalar.activation(
                    out=rrn[:], in_=ssq_all[:], func=AF.Sqrt
                )
                nc.vector.tensor_scalar_max(out=rrn[:], in0=rrn[:], scalar1=EPS_COS)
                nc.vector.reciprocal(out=rrn[:], in_=rrn[:])
                rsn = rrn[:, 0:T]
                rtn = rrn[:, T : 2 * T]
                mps_hold = None
                for t in range(T):
                    b = t // 2
                    sntn = []
                    for which, src in ((0, se_f), (1, tee_f)):
                        nt = rot.tile([128, D], BF16, tag="sntn")
                        nc.vector.tensor_scalar_mul(
                            out=nt[:], in0=src[:, t, :],
                            scalar1=rrn[:, which * T + t : which * T + t + 1],
                        )
                        sntn.append(nt)
                    if t % 2 == 0:
                        mps_hold = pmp.tile([128, D], F32, tag="mp")
                    nc.tensor.matmul(
                        mps_hold[:], sntn[0][:], sntn[1][:],
                        start=(t % 2 == 0), stop=(t % 2 == 1),
                    )
                    if t % 2 == 1:
                        nc.any.tensor_copy(out=m_sb[:, b, :], in_=mps_hold[:])

            # ---- AllReduce of M ---------------------------------------
            if section(7):
                m_in = dram.tile([B * D, D], F32, tag="m_in")
                m_out = dram.tile([B * D, D], F32, tag="m_out")
                nc.gpsimd.dma_start(
                    out=m_in[:].rearrange("(b p) d -> p b d", p=128), in_=m_sb[:]
                )
                nc.gpsimd.collective_compute(
                    "AllReduce",
                    OP.add,
                    replica_groups=[list(range(n_cores))],
                    ins=[m_in[:].opt()],
                    outs=[m_out[:].opt()],
                )
                m_all = big.tile([128, B, D], F32, tag="m_all")
                nc.gpsimd.dma_start(
                    out=m_all[:], in_=m_out[:].rearrange("(b p) d -> p b d", p=128)
                )
                m_bf = big.tile([128, B, D], BF16, tag="m_bf")
                for b in range(B):
                    nc.scalar.mul(out=m_bf[:, b, :], in_=m_all[:, b, :], mul=1.0 / S)
                g_bf = big.tile([128, B, D], BF16, tag="g_bf")
                for b in range(B):
                    ps = psm.tile([128, D], F32, tag="sm")
                    nc.tensor.matmul(ps[:], wsp[:], m_bf[:, b, :])
                    nc.any.tensor_copy(out=g_bf[:, b, :], in_=ps[:])

            # ---- local window attention + LN1 -------------------------
            if section(8):
                ln1 = big.tile([128, T, D], F32, tag="ln1")
                ln1T = big.tile([128, T, D], BF16, tag="ln1T")
                res1_all = big.tile([128, T, D], F32, tag="res1_all")
                mv1_all = big.tile([128, T, 2], F32, tag="mv1_all")
                std1_all = big.tile([128, T], F32, tag="std1_all")
                for t in range(T):
                    # one psum tile per chan-strip s4 (row group): 4 MMs each
                    # (par x window), same row-strip -> bank sharing is safe
                    sc4 = []
                    for s4 in range(4):
                        scp = p512.tile([128, 128], F32, tag="A", name=f"scp{s4}")
                        mm4 = [(par, wdw) for par in (0, 1) for wdw in (0, 1)]
                        for par, wdw in mm4:
                            qT_, kT_ = (qeT, keT) if par == 0 else (qoT, koT)
                            nc.tensor.matmul(
                                scp[64 * wdw : 64 * wdw + 64,
                                    64 * par : 64 * par + 64],
                                kT_[32 * s4 : 32 * s4 + 32, t,
                                    64 * wdw : 64 * wdw + 64],
                                qT_[32 * s4 : 32 * s4 + 32, t,
                                    64 * wdw : 64 * wdw + 64],
                                tile_position=(32 * s4, 64 * wdw),
                                start=(par == 0), stop=(par == 1),
                                skip_group_check=True,
                            )
                        sc4.append(scp)
                    if stage < 8.1:
                        continue
                    e_sb = rot.tile([128, 8 * W], BF16, tag="esb")
                    for s4 in range(4):
                        nc.scalar.activation(
                            out=e_sb[:, 128 * s4 : 128 * s4 + 128],
                            in_=sc4[s4][:], func=AF.Exp, scale=0.25,
                        )
                    nc.vector.tensor_mul(e_sb[:], e_sb[:], mask01[:])
                    if stage < 8.4:
                        continue
                    avp = []
                    for wdw in (0, 1):
                        avw = psm.tile([128, 8 * 17], F32, tag="sm", name=f"avw{wdw}")
                        ws = slice(64 * wdw, 64 * wdw + 64)
                        for h in range(8):
                            # e_sb col of head h = 128*(h//2) + 64*(h%2)
                            ec = 128 * (h // 2) + 64 * (h % 2)
                            nc.tensor.matmul(
                                avw[ws, 17 * h : 17 * h + 17],
                                e_sb[ws, ec : ec + W],
                                vaug[ws, t, 17 * h : 17 * h + 17],
                                start=(h == 0), stop=(h == 7),
                                skip_group_check=True,
                            )
                        avp.append(avw)
                    if stage < 8.6:
                        continue
                    av_sb = rot.tile([128, D], BF16, tag="avsb")
                    rden = st.tile([128, 8], F32, tag="rden")
                    for wdw in (0, 1):
                        ws = slice(64 * wdw, 64 * wdw + 64)
                        avv = avp[wdw][:].rearrange("p (h c) -> p h c", c=17)
                        nc.vector.reciprocal(out=rden[ws, :], in_=avv[ws, :, 16])
                        rd = rden[ws, :]
                        nc.vector.tensor_tensor(
                            out=av_sb[ws, :].rearrange("p (h c) -> p h c", c=16),
                            in0=avv[ws, :, 0:16],
                            in1=bass.AP(
                                tensor=rd.tensor, offset=rd.offset,
                                ap=[list(rd.ap[0]), list(rd.ap[1]), [0, 16]],
                            ),
                            op=OP.mult,
                        )
                    if stage < 8.8:
                        continue
                    avt_ps = psm.tile([128, D], BF16, tag="sm")
                    nc.tensor.transpose(avt_ps[:], av_sb[:], identb[:])
                    avT = rot.tile([128, D], BF16, tag="avT")
                    nc.any.tensor_copy(out=avT[:], in_=avt_ps[:])
                    ops_ = psm.tile([128, D], F32, tag="sm")
                    nc.tensor.matmul(ops_[:], avT[:], woutT[:])
                    nc.vector.tensor_add(res1_all[:, t, :], ops_[:], x_f[:, t, :])
                    stats = st.tile([128, 6], F32, tag="bst")
                    nc.vector.bn_stats(out=stats[:], in_=res1_all[:, t, :])
                    nc.vector.bn_aggr(out=mv1_all[:, t, :], in_=stats[:])
                # batched 1/sqrt(var+eps) for all tiles, then normalize
                nc.scalar.activation(
                    out=std1_all[:], in_=mv1_all[:, :, 1], func=AF.Sqrt,
                    bias=epsln[:],
                )
                nc.vector.reciprocal(out=std1_all[:], in_=std1_all[:])
                for t in range(T):
                    nc.vector.tensor_scalar(
                        out=ln1[:, t, :], in0=res1_all[:, t, :],
                        scalar1=mv1_all[:, t, 0:1], scalar2=std1_all[:, t : t + 1],
                        op0=OP.subtract, op1=OP.mult,
                    )
                    psT = psm.tile([128, D], F32, tag="sm")
                    nc.tensor.transpose(psT[:], ln1[:, t, :], identf[:])
                    nc.any.tensor_copy(out=ln1T[:, t, :], in_=psT[:])

            # ---- FFN (hT-direct) + LN2 --------------------------------
            if section(9):
                hT = big.tile([128, 4, T, D], BF16, tag="hT")
                for k4 in range(4):
                    for t4 in range(4):
                        sl = slice(4 * t4, 4 * t4 + 4)
                        ps = p512.tile([128, 512], F32, tag="A")
                        nc.tensor.matmul(ps[:], w1t[:, k4, :], ln1T[:, sl, :])
                        nc.scalar.activation(
                            out=hT[:, k4, sl, :].rearrange("p a b -> p (a b)"),
                            in_=ps[:], func=AF.Gelu,
                        )
                xm2 = big.tile([128, T, D], F32, tag="xm2")
                res2_all = big.tile([128, T, D], F32, tag="res2_all")
                mv2_all = big.tile([128, T, 2], F32, tag="mv2_all")
                std2_all = big.tile([128, T], F32, tag="std2_all")
                for t4 in range(4):
                    sl = slice(4 * t4, 4 * t4 + 4)
                    ps = p512.tile([128, 512], F32, tag="A")
                    for k4 in range(4):
                        nc.tensor.matmul(
                            ps[:], w2t[:, k4, :], hT[:, k4, sl, :],
                            start=(k4 == 0), stop=(k4 == 3),
                        )
                    o2T = rot.tile([128, 512], BF16, tag="o2T")
                    nc.any.tensor_copy(out=o2T[:], in_=ps[:])
                    for tt in range(4):
                        t = 4 * t4 + tt
                        tps = psm.tile([128, D], BF16, tag="sm")
                        nc.tensor.transpose(
                            tps[:], o2T[:, 128 * tt : 128 * tt + 128], identb[:]
                        )
                        nc.vector.tensor_add(
                            res2_all[:, t, :], tps[:], ln1[:, t, :]
                        )
                        stats = st.tile([128, 6], F32, tag="bst")
                        nc.vector.bn_stats(out=stats[:], in_=res2_all[:, t, :])
                        nc.vector.bn_aggr(out=mv2_all[:, t, :], in_=stats[:])
                nc.scalar.activation(
                    out=std2_all[:], in_=mv2_all[:, :, 1], func=AF.Sqrt,
                    bias=epsln[:],
                )
                nc.vector.reciprocal(out=std2_all[:], in_=std2_all[:])
                for t in range(T):
                    nc.vector.tensor_scalar(
                        out=xm2[:, t, :], in0=res2_all[:, t, :],
                        scalar1=mv2_all[:, t, 0:1], scalar2=std2_all[:, t : t + 1],
                        op0=OP.subtract, op1=OP.mult,
                    )

            # ---- interaction MHA over the batch axis (DVE) ------------
            if section(10):
                z_all = big.tile([128, 2, 8, 8, 8], F32, tag="z_all")
                for hi in range(2):
                    kv_view = k_int[:].rearrange(
                        "p (bb two) c -> p two bb c", two=2
                    )[:, hi]
                    for i in range(8):
                        qa = q_int[:, 2 * i + hi, :]
                        tmp = rot.tile([128, 8, D], BF16, tag="itmp")
                        nc.vector.tensor_tensor(
                            out=tmp[:],
                            in0=kv_view,
                            in1=bass.AP(
                                tensor=qa.tensor, offset=qa.offset,
                                ap=[list(qa.ap[0]), [0, 8], [1, D]],
                            ),
                            op=OP.mult,
                        )
                        zslice = z_all[:, hi, i, :, :]
                        zout = bass.AP(
                            tensor=zslice.tensor, offset=zslice.offset,
                            ap=[list(zslice.ap[0]), [1, 8], [8, 8]],
                        )
                        nc.vector.reduce_sum(
                            out=zout,
                            in_=tmp[:].rearrange("p j (h c) -> p j h c", c=16),
                            axis=mybir.AxisListType.X,
                        )
                e_all = big.tile([128, 2, 8, 8, 8], BF16, tag="e_all")
                nc.scalar.activation(
                    out=e_all[:].rearrange("p a b c d -> p (a b c d)"),
                    in_=z_all[:].rearrange("p a b c d -> p (a b c d)"),
                    func=AF.Exp, scale=0.25,
                )
                den = st.tile([128, 2, 8, 8], F32, tag="iden")
                nc.vector.reduce_sum(
                    out=den[:], in_=e_all[:], axis=mybir.AxisListType.X
                )
                nc.vector.reciprocal(
                    out=den[:].rearrange("p a b c -> p (a b c)"),
                    in_=den[:].rearrange("p a b c -> p (a b c)"),
                )
                en = big.tile([128, 2, 8, 8, 8], BF16, tag="en")
                dv = den[:].rearrange("p a b c -> p (a b c)")
                nc.vector.tensor_tensor(
                    out=en[:].rearrange("p a b c d -> p (a b c) d"),
                    in0=e_all[:].rearrange("p a b c d -> p (a b c) d"),
                    in1=bass.AP(
                        tensor=dv.tensor, offset=dv.offset,
                        ap=[list(dv.ap[0]), [1, 128], [0, 8]],
                    ),
                    op=OP.mult,
                )
                av_int = big.tile([128, T, D], BF16, tag="av_int")
                for hi in range(2):
                    v_view = v_int[:].rearrange(
                        "p (bb two) c -> p two bb c", two=2
                    )[:, hi]
                    for i in range(8):
                        asl = en[:, hi, i, :, :]
                        a_ap = bass.AP(
                            tensor=asl.tensor, offset=asl.offset,
                            ap=[list(asl.ap[0]), [1, 8], [8, 8], [0, 16]],
                        )
                        tmp = rot.tile([128, 8, D], BF16, tag="itmp")
                        nc.gpsimd.tensor_tensor(
                            out=tmp[:],
                            in0=v_view.rearrange("p j (h c) -> p j h c", c=16),
                            in1=a_ap, op=OP.mult,
                        )
                        # contiguous log-tree reduction over j (8 -> 1)
                        nc.vector.tensor_add(
                            tmp[:, 0:4, :].rearrange("p a b -> p (a b)"),
                            tmp[:, 0:4, :].rearrange("p a b -> p (a b)"),
                            tmp[:, 4:8, :].rearrange("p a b -> p (a b)"),
                        )
                        nc.vector.tensor_add(
                            tmp[:, 0:2, :].rearrange("p a b -> p (a b)"),
                            tmp[:, 0:2, :].rearrange("p a b -> p (a b)"),
                            tmp[:, 2:4, :].rearrange("p a b -> p (a b)"),
                        )
                        nc.vector.tensor_add(
                            av_int[:, 2 * i + hi, :], tmp[:, 0, :], tmp[:, 1, :]
                        )

            # ---- z / sim ----------------------------------------------
            if section(11):
                sim = big.tile([128, T], F32, tag="sim")
                for t in range(T):
                    b = t // 2
                    zps = psm.tile([128, D], F32, tag="sm")
                    nc.tensor.matmul(zps[:], spT[:, t, :], g_bf[:, b, :])
                    scratch = st.tile([128, D], F32, tag="zscr")
                    dot = st.tile([128, 1], F32, tag="zdot")
                    nc.vector.tensor_mul(scratch[:], zps[:], tee_f[:, t, :])
                    nc.vector.reduce_sum(
                        out=dot[:], in_=scratch[:], axis=mybir.AxisListType.X
                    )
                    nc.vector.tensor_scalar(
                        out=sim[:, t : t + 1], in0=dot[:],
                        scalar1=rsn[:, t : t + 1], scalar2=rtn[:, t : t + 1],
                        op0=OP.mult, op1=OP.mult,
                    )

            # ---- interaction out-proj + final combine -----------------
            if section(12):
                for t in range(T):
                    tps = psm.tile([128, D], BF16, tag="sm")
                    nc.tensor.transpose(tps[:], av_int[:, t, :], identb[:])
                    avIT = rot.tile([128, D], BF16, tag="avIT")
                    nc.any.tensor_copy(out=avIT[:], in_=tps[:])
                    ips = psm.tile([128, D], F32, tag="sm")
                    nc.tensor.matmul(ips[:], avIT[:], woiT[:])
                    outt = rot.tile([128, D], F32, tag="outt")
                    nc.vector.tensor_scalar_mul(
                        out=outt[:], in0=ips[:], scalar1=sim[:, t : t + 1]
                    )
                    nc.vector.tensor_add(outt[:], outt[:], xm2[:, t, :])
                    nc.sync.dma_start(out=out_t[:, t, :], in_=outt[:])

    nc.compile()
    return nc


def _prep_host(inputs):
    """Host-side weight folding / permutation. Returns dict of device arrays."""
    f32 = np.float32
    bf = ml_dtypes.bfloat16
    g = {k: np.asarray(v, f32) for k, v in inputs.items()}

    for nm in ("lw_in_b", "lw_out_b", "spat_b", "temp_b", "int_in_b",
               "int_out_b", "ffn_b1", "ffn_b2", "ln1_b", "ln2_b"):
        assert not np.any(g[nm]), f"nonzero bias {nm} unsupported"
    assert np.all(g["ln1_g"] == 1.0) and np.all(g["ln2_g"] == 1.0), "ln gamma"

    Wq, Wk, Wv = g["lw_in_w"][:D], g["lw_in_w"][D:2*D], g["lw_in_w"][2*D:]

    def padT(Wm, par):
        out = np.zeros((D, D), f32)
        for s4 in range(4):
            h = 2 * s4 + par
            out[32 * s4 : 32 * s4 + 16, :] = Wm[16 * h : 16 * h + 16, :]
        return np.ascontiguousarray(out.T)

    wvaug = np.zeros((D, 8 * 17), f32)
    for h in range(8):
        wvaug[:, 17 * h : 17 * h + 16] = Wv[16 * h : 16 * h + 16, :].T

    mask01 = np.zeros((D, 8 * W), f32)
    jj = np.arange(D) % W
    ii = np.arange(8 * W) % W
    mask01[:, :] = (jj[:, None] <= ii[None, :])

    WqI = g["int_in_w"][:D]
    WkI = g["int_in_w"][D:2*D]
    WvI = g["int_in_w"][2*D:]

    arrs = {
        "wqte": padT(Wq, 0), "wqto": padT(Wq, 1),
        "wkte": padT(Wk, 0), "wkto": padT(Wk, 1),
        "wvaug": wvaug,
        "woutT": g["lw_out_w"].T,
        "w1t": g["ffn_w1"].T.reshape(D, 4, D),
        "w2t": g["ffn_w2"].T.reshape(4, D, D).transpose(1, 0, 2),
        "wsqT": np.concatenate([g["spat_w"].T, (WqI @ g["spat_w"]).T], axis=1),
        "wtkvT": np.concatenate(
            [g["temp_w"].T, (WkI @ g["temp_w"]).T, (WvI @ g["temp_w"]).T], axis=1
        ),
        "wsp": g["spat_w"],
        "woiT": g["int_out_w"].T,
        "mask01T": mask01,
        "identb": np.eye(D, dtype=f32),
    }
    out = {k: np.ascontiguousarray(v.astype(bf)) for k, v in arrs.items()}
    out["identf"] = np.ascontiguousarray(np.eye(D, dtype=f32))
    return out


def kernel(x, spatial_info, temporal_info, **weights):
    global _last_in_maps
    inputs = dict(weights)
    x = np.ascontiguousarray(np.asarray(x, np.float32))
    sp = np.ascontiguousarray(np.asarray(spatial_info, np.float32))
    te = np.ascontiguousarray(np.asarray(temporal_info, np.float32))

    if "nc" not in _BUILD_CACHE:
        _BUILD_CACHE["nc"] = _build(NCORE)
    nc = _BUILD_CACHE["nc"]

    host = _prep_host(inputs)
    in_maps = []
    for c in range(NCORE):
        sl = slice(SP * c, SP * c + SP)
        m = dict(host)
        m["xc"] = np.ascontiguousarray(x[:, sl, :].reshape(R, D))
        m["spc"] = np.ascontiguousarray(sp[:, sl, :].reshape(R, D))
        m["tec"] = np.ascontiguousarray(te[:, sl, :].reshape(R, D))
        in_maps.append(m)
    _last_in_maps = in_maps

    res = run_bass_kernel_spmd(nc, in_maps, list(range(NCORE)))
    out = np.empty((B, S, D), np.float32)
    for c in range(NCORE):
        out[:, SP * c : SP * c + SP, :] = res.results[c]["outc"].reshape(B, SP, D)
    return out


# revision 22
# speedup vs baseline: 1.4081x; 1.1160x over previous
"""Trainium2 Bass kernel for nn_EnhancedTransformer_15350213116361.

Sequence-parallel across 8 NeuronCores: core c owns positions
[256c, 256c+256) of ALL batches (rows r = b*256 + s_local, 2048 rows/core).
Window attention (W=64) is position-local, FFN/LN/projections are row-local,
and the cross-batch interaction MHA needs only same-position rows, which a
sequence shard keeps together.  The only cross-core data dependency is the
per-batch Gram matrix M_b = sn_b^T tn_b (summed over all S), handled with a
single 512 KB AllReduce.

Layout: rows on partitions, [128p, 16 tiles, chan] sbuf tensors; matmuls are
fed by PE-transposed stationaries or transposed weights; compute in bf16
(tolerance gate is 2e-2), accumulation in f32.
"""

import sys

sys.path.insert(0, "/opt/trn_rl_repo")

import numpy as np
import ml_dtypes

import concourse.bass as bass
import concourse.bacc as bacc
import concourse.tile as tile
from concourse import mybir
from concourse.bass_utils import run_bass_kernel_spmd

BF16 = mybir.dt.bfloat16
F32 = mybir.dt.float32
AF = mybir.ActivationFunctionType
OP = mybir.AluOpType

B, S, D, H, W = 8, 2048, 128, 8, 64
NCORE = 8
SP = S // NCORE          # positions per core
R = B * SP               # rows per core
T = R // 128             # 128-row tiles per core
EPS_LN = 1e-5
EPS_COS = 1e-8

_BUILD_CACHE = {}
_last_in_maps = None


def _build(n_cores: int, stage: float = 99):
    """Build the per-core SPMD Bass program (same program on every core).

    `stage` truncates the program for bisection (99 = full kernel).
    """
    nc = bacc.Bacc(None, target_bir_lowering=False)

    # ---- DRAM I/O ------------------------------------------------------
    xc_d = nc.dram_tensor("xc", [R, D], F32, kind="ExternalInput")
    spc_d = nc.dram_tensor("spc", [R, D], F32, kind="ExternalInput")
    tec_d = nc.dram_tensor("tec", [R, D], F32, kind="ExternalInput")

    def win(name, shape, dt=BF16):
        return nc.dram_tensor(name, shape, dt, kind="ExternalInput")

    wqte_d = win("wqte", [D, D])
    wqto_d = win("wqto", [D, D])
    wkte_d = win("wkte", [D, D])
    wkto_d = win("wkto", [D, D])
    wvaug_d = win("wvaug", [D, 8 * 17])
    wout_d = win("woutT", [D, D])
    w1t_d = win("w1t", [D, 4, D])
    w2t_d = win("w2t", [D, 4, D])
    wsq_d = win("wsqT", [D, 2 * D])
    wtkv_d = win("wtkvT", [D, 3 * D])
    wsp_d = win("wsp", [D, D])
    woit_d = win("woiT", [D, D])
    mask_d = win("mask01T", [D, 8 * W])
    identb_d = win("identb", [D, D])
    identf_d = win("identf", [D, D], F32)

    out_d = nc.dram_tensor("outc", [R, D], F32, kind="ExternalOutput")

    xc_t = xc_d.rearrange("(t p) d -> p t d", p=128)
    spc_t = spc_d.rearrange("(t p) d -> p t d", p=128)
    tec_t = tec_d.rearrange("(t p) d -> p t d", p=128)
    out_t = out_d.rearrange("(t p) d -> p t d", p=128)

    with tile.TileContext(nc) as tc:
        with (
            tc.tile_pool(name="big", bufs=1) as big,          # persistent sbuf
            tc.tile_pool(name="wts", bufs=1) as wts,          # weights/constants
            tc.tile_pool(name="rot", bufs=4) as rot,          # rotating sbuf tiles
            tc.tile_pool(name="st", bufs=4) as st,            # small stats tiles
            tc.tile_pool(name="p512", bufs=3, space="PSUM") as p512,
            tc.tile_pool(name="psm", bufs=4, space="PSUM") as psm,
            tc.tile_pool(name="pmp", bufs=1, space="PSUM") as pmp,
            tc.tile_pool(name="dram", bufs=1, space="DRAM") as dram,
        ):
            # ---- weight / const loads ---------------------------------
            def wtile(dram_t, shape, dt=BF16):
                t_ = wts.tile(shape, dt, tag=dram_t.name)
                nc.sync.dma_start(out=t_[:], in_=dram_t[:])
                return t_

            wqte = wtile(wqte_d, [D, D])
            wqto = wtile(wqto_d, [D, D])
            wkte = wtile(wkte_d, [D, D])
            wkto = wtile(wkto_d, [D, D])
            wvaug = wtile(wvaug_d, [D, 8 * 17])
            woutT = wtile(wout_d, [D, D])
            w1t = wtile(w1t_d, [D, 4, D])
            w2t = wtile(w2t_d, [D, 4, D])
            wsqT = wtile(wsq_d, [D, 2 * D])
            wtkvT = wtile(wtkv_d, [D, 3 * D])
            wsp = wtile(wsp_d, [D, D])
            woiT = wtile(woit_d, [D, D])
            mask01 = wtile(mask_d, [D, 8 * W])
            identb = wtile(identb_d, [D, D])
            identf = wtile(identf_d, [D, D], F32)

            epsln = wts.tile([128, 1], F32, tag="epsln")
            nc.vector.memset(epsln[:], EPS_LN)

            # ---- input loads ------------------------------------------
            x_f = big.tile([128, T, D], F32, tag="x_f")
            sp_f = big.tile([128, T, D], F32, tag="sp_f")
            te_raw = big.tile([128, T, D], F32, tag="te_raw")
            for q in range(4):
                sl = slice(4 * q, 4 * q + 4)
                nc.sync.dma_start(out=x_f[:, sl, :], in_=xc_t[:, sl, :])
                nc.sync.dma_start(out=sp_f[:, sl, :], in_=spc_t[:, sl, :])
                nc.sync.dma_start(out=te_raw[:, sl, :], in_=tec_t[:, sl, :])

            def section(n):
                return stage >= n

            # ---- transposes of x / spatial / temporal -----------------
            if section(2):
                xT = big.tile([128, T, D], BF16, tag="xT")
                spT = big.tile([128, T, D], BF16, tag="spT")
                teT = big.tile([128, T, D], BF16, tag="teT")
                for t in range(T):
                    for src, dst in ((x_f, xT), (sp_f, spT), (te_raw, teT)):
                        ps = psm.tile([128, D], F32, tag="sm")
                        nc.tensor.transpose(ps[:], src[:, t, :], identf[:])
                        nc.any.tensor_copy(out=dst[:, t, :], in_=ps[:])

            # ---- padded q/k projections (W-stationary, xT moving) -----
            if section(3):
                qeT = big.tile([128, T, D], BF16, tag="qeT")
                qoT = big.tile([128, T, D], BF16, tag="qoT")
                keT = big.tile([128, T, D], BF16, tag="keT")
                koT = big.tile([128, T, D], BF16, tag="koT")
                for wmat, dst in ((wqte, qeT), (wqto, qoT), (wkte, keT), (wkto, koT)):
                    for q in range(4):
                        sl = slice(4 * q, 4 * q + 4)
                        ps = p512.tile([128, 512], F32, tag="A")
                        nc.tensor.matmul(ps[:], wmat[:], xT[:, sl, :])
                        nc.any.tensor_copy(
                            out=dst[:, sl, :],
                            in_=ps[:].rearrange("p (a b) -> p a b", a=4),
                        )

            # ---- v_aug projection (xT tile stationary) ----------------
            if section(4):
                vaug = big.tile([128, T, 8 * 17], BF16, tag="vaug")
                for t in range(T):
                    ps = psm.tile([128, 8 * 17], F32, tag="sm")
                    nc.tensor.matmul(ps[:], xT[:, t, :], wvaug[:])
                    nc.any.tensor_copy(out=vaug[:, t, :], in_=ps[:])
                    nc.vector.memset(
                        vaug[:, t, :].rearrange("p (h c) -> p h c", c=17)[:, :, 16],
                        1.0,
                    )

            # ---- se|q_int and te|k_int|v_int projections --------------
            if section(5):
                seq = big.tile([128, T, 2 * D], BF16, tag="seq")
                tkv = big.tile([128, T, 3 * D], BF16, tag="tkv")
                for t in range(T):
                    ps = p512.tile([128, 2 * D], F32, tag="A")
                    nc.tensor.matmul(ps[:], spT[:, t, :], wsqT[:])
                    nc.any.tensor_copy(out=seq[:, t, :], in_=ps[:])
                    ps2 = p512.tile([128, 3 * D], F32, tag="A")
                    nc.tensor.matmul(ps2[:], teT[:, t, :], wtkvT[:])
                    nc.any.tensor_copy(out=tkv[:, t, :], in_=ps2[:])
                se_f = seq[:].rearrange("p t (a d) -> p t a d", a=2)[:, :, 0, :]
                q_int = seq[:].rearrange("p t (a d) -> p t a d", a=2)[:, :, 1, :]
                tee_f = tkv[:].rearrange("p t (a d) -> p t a d", a=3)[:, :, 0, :]
                k_int = tkv[:].rearrange("p t (a d) -> p t a d", a=3)[:, :, 1, :]
                v_int = tkv[:].rearrange("p t (a d) -> p t a d", a=3)[:, :, 2, :]

            # ---- cosine norms, sn/tn, M partials ----------------------
            if section(6):
                rrn = big.tile([128, 2 * T], F32, tag="rrn")  # [rsn | rtn]
                m_sb = big.tile([128, B, D], BF16, tag="m_sb")
                ssq_all = st.tile([128, 2 * T], F32, tag="ssq")
                for t in range(T):
                    for which, src in ((0, se_f), (1, tee_f)):
                        sq = st.tile([128, D], BF16, tag="sq")
                        nc.scalar.activation(
                            out=sq[:], in_=src[:, t, :], func=AF.Square,
                            accum_out=ssq_all[:, which * T + t : which * T + t + 1],
                        )
                nc.scalar.activation(
                    out=rrn[:], in_=ssq_all[:], func=AF.Sqrt
                )
                nc.vector.tensor_scalar_max(out=rrn[:], in0=rrn[:], scalar1=EPS_COS)
                nc.vector.reciprocal(out=rrn[:], in_=rrn[:])
                rsn = rrn[:, 0:T]
                rtn = rrn[:, T : 2 * T]
                mps_hold = None
                for t in range(T):
                    b = t // 2
                    sntn = []
                    for which, src in ((0, se_f), (1, tee_f)):
                        nt = rot.tile([128, D], BF16, tag="sntn")
                        nc.vector.tensor_scalar_mul(
                            out=nt[:], in0=src[:, t, :],
                            scalar1=rrn[:, which * T + t : which * T + t + 1],
                        )
                        sntn.append(nt)
                    if t % 2 == 0:
                        mps_hold = pmp.tile([128, D], F32, tag="mp")
                    nc.tensor.matmul(
                        mps_hold[:], sntn[0][:], sntn[1][:],
                        start=(t % 2 == 0), stop=(t % 2 == 1),
                    )
                    if t % 2 == 1:
                        nc.any.tensor_copy(out=m_sb[:, b, :], in_=mps_hold[:])

            # ---- AllReduce of M (bf16; 1/S folded into wsp host-side) --
            if section(7):
                m_in = dram.tile([B * D, D], BF16, tag="m_in")
                m_out = dram.tile([B * D, D], BF16, tag="m_out")
                nc.gpsimd.dma_start(
                    out=m_in[:].rearrange("(b p) d -> p b d", p=128), in_=m_sb[:]
                )
                nc.gpsimd.collective_compute(
                    "AllReduce",
                    OP.add,
                    replica_groups=[list(range(n_cores))],
                    ins=[m_in[:].opt()],
                    outs=[m_out[:].opt()],
                )
                m_bf = big.tile([128, B, D], BF16, tag="m_bf")
                nc.gpsimd.dma_start(
                    out=m_bf[:], in_=m_out[:].rearrange("(b p) d -> p b d", p=128)
                )
                g_bf = big.tile([128, B, D], BF16, tag="g_bf")
                for b in range(B):
                    ps = psm.tile([128, D], F32, tag="sm")
                    nc.tensor.matmul(ps[:], wsp[:], m_bf[:, b, :])
                    nc.any.tensor_copy(out=g_bf[:, b, :], in_=ps[:])

            # ---- local window attention + LN1 -------------------------
            if section(8):
                ln1 = big.tile([128, T, D], F32, tag="ln1")
                ln1T = big.tile([128, T, D], BF16, tag="ln1T")
                res1_all = big.tile([128, T, D], F32, tag="res1_all")
                mv1_all = big.tile([128, T, 2], F32, tag="mv1_all")
                std1_all = big.tile([128, T], F32, tag="std1_all")
                for t in range(T):
                    # one psum tile per chan-strip s4 (row group): 4 MMs each
                    # (par x window), same row-strip -> bank sharing is safe
                    sc4 = []
                    for s4 in range(4):
                        scp = p512.tile([128, 128], F32, tag="A", name=f"scp{s4}")
                        mm4 = [(par, wdw) for par in (0, 1) for wdw in (0, 1)]
                        for par, wdw in mm4:
                            qT_, kT_ = (qeT, keT) if par == 0 else (qoT, koT)
                            nc.tensor.matmul(
                                scp[64 * wdw : 64 * wdw + 64,
                                    64 * par : 64 * par + 64],
                                kT_[32 * s4 : 32 * s4 + 32, t,
                                    64 * wdw : 64 * wdw + 64],
                                qT_[32 * s4 : 32 * s4 + 32, t,
                                    64 * wdw : 64 * wdw + 64],
                                tile_position=(32 * s4, 64 * wdw),
                                start=(par == 0), stop=(par == 1),
                                skip_group_check=True,
                            )
                        sc4.append(scp)
                    if stage < 8.1:
                        continue
                    e_sb = rot.tile([128, 8 * W], BF16, tag="esb")
                    for s4 in range(4):
                        nc.scalar.activation(
                            out=e_sb[:, 128 * s4 : 128 * s4 + 128],
                            in_=sc4[s4][:], func=AF.Exp, scale=0.25,
                        )
                    nc.vector.tensor_mul(e_sb[:], e_sb[:], mask01[:])
                    if stage < 8.4:
                        continue
                    avp = []
                    for wdw in (0, 1):
                        avw = psm.tile([128, 8 * 17], F32, tag="sm", name=f"avw{wdw}")
                        ws = slice(64 * wdw, 64 * wdw + 64)
                        for h in range(8):
                            # e_sb col of head h = 128*(h//2) + 64*(h%2)
                            ec = 128 * (h // 2) + 64 * (h % 2)
                            nc.tensor.matmul(
                                avw[ws, 17 * h : 17 * h + 17],
                                e_sb[ws, ec : ec + W],
                                vaug[ws, t, 17 * h : 17 * h + 17],
                                start=(h == 0), stop=(h == 7),
                                skip_group_check=True,
                            )
                        avp.append(avw)
                    if stage < 8.6:
                        continue
                    av_sb = rot.tile([128, D], BF16, tag="avsb")
                    rden = st.tile([128, 8], F32, tag="rden")
                    for wdw in (0, 1):
                        ws = slice(64 * wdw, 64 * wdw + 64)
                        avv = avp[wdw][:].rearrange("p (h c) -> p h c", c=17)
                        nc.vector.reciprocal(out=rden[ws, :], in_=avv[ws, :, 16])
                        rd = rden[ws, :]
                        nc.vector.tensor_tensor(
                            out=av_sb[ws, :].rearrange("p (h c) -> p h c", c=16),
                            in0=avv[ws, :, 0:16],
                            in1=bass.AP(
                                tensor=rd.tensor, offset=rd.offset,
                                ap=[list(rd.ap[0]), list(rd.ap[1]), [0, 16]],
                            ),
                            op=OP.mult,
                        )
                    if stage < 8.8:
                        continue
                    avt_ps = psm.tile([128, D], BF16, tag="sm")
                    nc.tensor.transpose(avt_ps[:], av_sb[:], identb[:])
                    avT = rot.tile([128, D], BF16, tag="avT")
                    nc.any.tensor_copy(out=avT[:], in_=avt_ps[:])
                    ops_ = psm.tile([128, D], F32, tag="sm")
                    nc.tensor.matmul(ops_[:], avT[:], woutT[:])
                    nc.vector.tensor_add(res1_all[:, t, :], ops_[:], x_f[:, t, :])
                    stats = st.tile([128, 6], F32, tag="bst")
                    nc.vector.bn_stats(out=stats[:], in_=res1_all[:, t, :])
                    nc.vector.bn_aggr(out=mv1_all[:, t, :], in_=stats[:])
                # batched 1/sqrt(var+eps) for all tiles, then normalize
                nc.scalar.activation(
                    out=std1_all[:], in_=mv1_all[:, :, 1], func=AF.Sqrt,
                    bias=epsln[:],
                )
                nc.vector.reciprocal(out=std1_all[:], in_=std1_all[:])
                for t in range(T):
                    nc.vector.tensor_scalar(
                        out=ln1[:, t, :], in0=res1_all[:, t, :],
                        scalar1=mv1_all[:, t, 0:1], scalar2=std1_all[:, t : t + 1],
                        op0=OP.subtract, op1=OP.mult,
                    )
                    psT = psm.tile([128, D], F32, tag="sm")
                    nc.tensor.transpose(psT[:], ln1[:, t, :], identf[:])
                    nc.any.tensor_copy(out=ln1T[:, t, :], in_=psT[:])

            # ---- FFN (hT-direct) + LN2 --------------------------------
            if section(9):
                hT = big.tile([128, 4, T, D], BF16, tag="hT")
                for k4 in range(4):
                    for t4 in range(4):
                        sl = slice(4 * t4, 4 * t4 + 4)
                        ps = p512.tile([128, 512], F32, tag="A")
                        nc.tensor.matmul(ps[:], w1t[:, k4, :], ln1T[:, sl, :])
                        nc.scalar.activation(
                            out=hT[:, k4, sl, :].rearrange("p a b -> p (a b)"),
                            in_=ps[:], func=AF.Gelu,
                        )
                xm2 = big.tile([128, T, D], F32, tag="xm2")
                res2_all = big.tile([128, T, D], F32, tag="res2_all")
                mv2_all = big.tile([128, T, 2], F32, tag="mv2_all")
                std2_all = big.tile([128, T], F32, tag="std2_all")
                for t4 in range(4):
                    sl = slice(4 * t4, 4 * t4 + 4)
                    ps = p512.tile([128, 512], F32, tag="A")
                    for k4 in range(4):
                        nc.tensor.matmul(
                            ps[:], w2t[:, k4, :], hT[:, k4, sl, :],
                            start=(k4 == 0), stop=(k4 == 3),
                        )
                    o2T = rot.tile([128, 512], BF16, tag="o2T")
                    nc.any.tensor_copy(out=o2T[:], in_=ps[:])
                    for tt in range(4):
                        t = 4 * t4 + tt
                        tps = psm.tile([128, D], BF16, tag="sm")
                        nc.tensor.transpose(
                            tps[:], o2T[:, 128 * tt : 128 * tt + 128], identb[:]
                        )
                        nc.vector.tensor_add(
                            res2_all[:, t, :], tps[:], ln1[:, t, :]
                        )
                        stats = st.tile([128, 6], F32, tag="bst")
                        nc.vector.bn_stats(out=stats[:], in_=res2_all[:, t, :])
                        nc.vector.bn_aggr(out=mv2_all[:, t, :], in_=stats[:])
                nc.scalar.activation(
                    out=std2_all[:], in_=mv2_all[:, :, 1], func=AF.Sqrt,
                    bias=epsln[:],
                )
                nc.vector.reciprocal(out=std2_all[:], in_=std2_all[:])
                for t in range(T):
                    nc.vector.tensor_scalar(
                        out=xm2[:, t, :], in0=res2_all[:, t, :],
                        scalar1=mv2_all[:, t, 0:1], scalar2=std2_all[:, t : t + 1],
                        op0=OP.subtract, op1=OP.mult,
                    )

            # ---- interaction MHA over the batch axis (DVE) ------------
            if section(10):
                z_all = big.tile([128, 2, 8, 8, 8], F32, tag="z_all")
                for hi in range(2):
                    kv_view = k_int[:].rearrange(
                        "p (bb two) c -> p two bb c", two=2
                    )[:, hi]
                    for i in range(8):
                        qa = q_int[:, 2 * i + hi, :]
                        tmp = rot.tile([128, 8, D], BF16, tag="itmp")
                        nc.vector.tensor_tensor(
                            out=tmp[:],
                            in0=kv_view,
                            in1=bass.AP(
                                tensor=qa.tensor, offset=qa.offset,
                                ap=[list(qa.ap[0]), [0, 8], [1, D]],
                            ),
                            op=OP.mult,
                        )
                        zslice = z_all[:, hi, i, :, :]
                        zout = bass.AP(
                            tensor=zslice.tensor, offset=zslice.offset,
                            ap=[list(zslice.ap[0]), [1, 8], [8, 8]],
                        )
                        nc.vector.reduce_sum(
                            out=zout,
                            in_=tmp[:].rearrange("p j (h c) -> p j h c", c=16),
                            axis=mybir.AxisListType.X,
                        )
                e_all = big.tile([128, 2, 8, 8, 8], BF16, tag="e_all")
                nc.scalar.activation(
                    out=e_all[:].rearrange("p a b c d -> p (a b c d)"),
                    in_=z_all[:].rearrange("p a b c d -> p (a b c d)"),
                    func=AF.Exp, scale=0.25,
                )
                den = st.tile([128, 2, 8, 8], F32, tag="iden")
                nc.vector.reduce_sum(
                    out=den[:], in_=e_all[:], axis=mybir.AxisListType.X
                )
                nc.vector.reciprocal(
                    out=den[:].rearrange("p a b c -> p (a b c)"),
                    in_=den[:].rearrange("p a b c -> p (a b c)"),
                )
                en = big.tile([128, 2, 8, 8, 8], BF16, tag="en")
                dv = den[:].rearrange("p a b c -> p (a b c)")
                nc.vector.tensor_tensor(
                    out=en[:].rearrange("p a b c d -> p (a b c) d"),
                    in0=e_all[:].rearrange("p a b c d -> p (a b c) d"),
                    in1=bass.AP(
                        tensor=dv.tensor, offset=dv.offset,
                        ap=[list(dv.ap[0]), [1, 128], [0, 8]],
                    ),
                    op=OP.mult,
                )
                av_int = big.tile([128, T, D], BF16, tag="av_int")
                for hi in range(2):
                    v_view = v_int[:].rearrange(
                        "p (bb two) c -> p two bb c", two=2
                    )[:, hi]
                    for i in range(8):
                        asl = en[:, hi, i, :, :]
                        a_ap = bass.AP(
                            tensor=asl.tensor, offset=asl.offset,
                            ap=[list(asl.ap[0]), [1, 8], [8, 8], [0, 16]],
                        )
                        tmp = rot.tile([128, 8, D], BF16, tag="itmp")
                        nc.gpsimd.tensor_tensor(
                            out=tmp[:],
                            in0=v_view.rearrange("p j (h c) -> p j h c", c=16),
                            in1=a_ap, op=OP.mult,
                        )
                        # contiguous log-tree reduction over j (8 -> 1)
                        nc.vector.tensor_add(
                            tmp[:, 0:4, :].rearrange("p a b -> p (a b)"),
                            tmp[:, 0:4, :].rearrange("p a b -> p (a b)"),
                            tmp[:, 4:8, :].rearrange("p a b -> p (a b)"),
                        )
                        nc.vector.tensor_add(
                            tmp[:, 0:2, :].rearrange("p a b -> p (a b)"),
                            tmp[:, 0:2, :].rearrange("p a b -> p (a b)"),
                            tmp[:, 2:4, :].rearrange("p a b -> p (a b)"),
                        )
                        nc.vector.tensor_add(
                            av_int[:, 2 * i + hi, :], tmp[:, 0, :], tmp[:, 1, :]
                        )

            # ---- z / sim ----------------------------------------------
            if section(11):
                sim = big.tile([128, T], F32, tag="sim")
                for t in range(T):
                    b = t // 2
                    zps = psm.tile([128, D], F32, tag="sm")
                    nc.tensor.matmul(zps[:], spT[:, t, :], g_bf[:, b, :])
                    scratch = st.tile([128, D], F32, tag="zscr")
                    dot = st.tile([128, 1], F32, tag="zdot")
                    nc.vector.tensor_mul(scratch[:], zps[:], tee_f[:, t, :])
                    nc.vector.reduce_sum(
                        out=dot[:], in_=scratch[:], axis=mybir.AxisListType.X
                    )
                    nc.vector.tensor_scalar(
                        out=sim[:, t : t + 1], in0=dot[:],
                        scalar1=rsn[:, t : t + 1], scalar2=rtn[:, t : t + 1],
                        op0=OP.mult, op1=OP.mult,
                    )

            # ---- interaction out-proj + final combine -----------------
            if section(12):
                for t in range(T):
                    tps = psm.tile([128, D], BF16, tag="sm")
                    nc.tensor.transpose(tps[:], av_int[:, t, :], identb[:])
                    avIT = rot.tile([128, D], BF16, tag="avIT")
                    nc.any.tensor_copy(out=avIT[:], in_=tps[:])
                    ips = psm.tile([128, D], F32, tag="sm")
                    nc.tensor.matmul(ips[:], avIT[:], woiT[:])
                    outt = rot.tile([128, D], F32, tag="outt")
                    nc.vector.tensor_scalar_mul(
                        out=outt[:], in0=ips[:], scalar1=sim[:, t : t + 1]
                    )
                    nc.vector.tensor_add(outt[:], outt[:], xm2[:, t, :])
                    nc.sync.dma_start(out=out_t[:, t, :], in_=outt[:])

    nc.compile()
    return nc


def _prep_host(inputs):
    """Host-side weight folding / permutation. Returns dict of device arrays."""
    f32 = np.float32
    bf = ml_dtypes.bfloat16
    g = {k: np.asarray(v, f32) for k, v in inputs.items()}

    for nm in ("lw_in_b", "lw_out_b", "spat_b", "temp_b", "int_in_b",
               "int_out_b", "ffn_b1", "ffn_b2", "ln1_b", "ln2_b"):
        assert not np.any(g[nm]), f"nonzero bias {nm} unsupported"
    assert np.all(g["ln1_g"] == 1.0) and np.all(g["ln2_g"] == 1.0), "ln gamma"

    Wq, Wk, Wv = g["lw_in_w"][:D], g["lw_in_w"][D:2*D], g["lw_in_w"][2*D:]

    def padT(Wm, par):
        out = np.zeros((D, D), f32)
        for s4 in range(4):
            h = 2 * s4 + par
            out[32 * s4 : 32 * s4 + 16, :] = Wm[16 * h : 16 * h + 16, :]
        return np.ascontiguousarray(out.T)

    wvaug = np.zeros((D, 8 * 17), f32)
    for h in range(8):
        wvaug[:, 17 * h : 17 * h + 16] = Wv[16 * h : 16 * h + 16, :].T

    mask01 = np.zeros((D, 8 * W), f32)
    jj = np.arange(D) % W
    ii = np.arange(8 * W) % W
    mask01[:, :] = (jj[:, None] <= ii[None, :])

    WqI = g["int_in_w"][:D]
    WkI = g["int_in_w"][D:2*D]
    WvI = g["int_in_w"][2*D:]

    arrs = {
        "wqte": padT(Wq, 0), "wqto": padT(Wq, 1),
        "wkte": padT(Wk, 0), "wkto": padT(Wk, 1),
        "wvaug": wvaug,
        "woutT": g["lw_out_w"].T,
        "w1t": g["ffn_w1"].T.reshape(D, 4, D),
        "w2t": g["ffn_w2"].T.reshape(4, D, D).transpose(1, 0, 2),
        "wsqT": np.concatenate([g["spat_w"].T, (WqI @ g["spat_w"]).T], axis=1),
        "wtkvT": np.concatenate(
            [g["temp_w"].T, (WkI @ g["temp_w"]).T, (WvI @ g["temp_w"]).T], axis=1
        ),
        "wsp": g["spat_w"] / np.float32(S),
        "woiT": g["int_out_w"].T,
        "mask01T": mask01,
        "identb": np.eye(D, dtype=f32),
    }
    out = {k: np.ascontiguousarray(v.astype(bf)) for k, v in arrs.items()}
    out["identf"] = np.ascontiguousarray(np.eye(D, dtype=f32))
    return out


def kernel(x, spatial_info, temporal_info, **weights):
    global _last_in_maps
    inputs = dict(weights)
    x = np.ascontiguousarray(np.asarray(x, np.float32))
    sp = np.ascontiguousarray(np.asarray(spatial_info, np.float32))
    te = np.ascontiguousarray(np.asarray(temporal_info, np.float32))

    if "nc" not in _BUILD_CACHE:
        _BUILD_CACHE["nc"] = _build(NCORE)
    nc = _BUILD_CACHE["nc"]

    host = _prep_host(inputs)
    in_maps = []
    for c in range(NCORE):
        sl = slice(SP * c, SP * c + SP)
        m = dict(host)
        m["xc"] = np.ascontiguousarray(x[:, sl, :].reshape(R, D))
        m["spc"] = np.ascontiguousarray(sp[:, sl, :].reshape(R, D))
        m["tec"] = np.ascontiguousarray(te[:, sl, :].reshape(R, D))
        in_maps.append(m)
    _last_in_maps = in_maps

    res = run_bass_kernel_spmd(nc, in_maps, list(range(NCORE)))
    out = np.empty((B, S, D), np.float32)
    for c in range(NCORE):
        out[:, SP * c : SP * c + SP, :] = res.results[c]["outc"].reshape(B, SP, D)
    return out
